# revision 4
# baseline (speedup 1.0000x reference)
"""DKVMN Bass kernel for Trainium2 (8 NeuronCores, data-parallel over batch).

Math (per batch item b, per step t):
    kn    = W_k[idx] + b_k
    alpha = softmax(kn @ km.T)                       [N]
    rt    = sum_n alpha[n] * h[n, :]                 [H]
    ft    = tanh([rt, kn] @ W_ft + b_ft);  pred = sigmoid(ft @ W_s + b_s)
    vt    = W_cks[idx] + sc * W_cks[idx+K] + b_cks
    et    = sigmoid(vt @ W_e + b_e);  at = tanh(vt @ W_a + b_a)
    h     = h * (1 - alpha (x) et) + alpha (x) at

Key restructuring:
  * All per-index quantities are gathered from HOST-premultiplied tables:
      G   = W_k @ km.T   (+ b_k@km.T)          -> softmax logits rows
      Elo/Ehi = W_cks[:K]@W_e / W_cks[K:]@W_e  (+ bias fold)
      Alo/Ahi = W_cks[:K]@W_a / W_cks[K:]@W_a  (+ bias fold)
      F   = W_k @ W_ft[H:] (+ b_k@W_ft[H:] + b_ft)
    packed as one row-gatherable table Z [K, 370].
  * The recurrent state h is SBUF-resident in layout h[b, j*64+n] (j=hidden,
    n=slot).  Per step, with NEGA = -alpha:
      m1 = h * NEGA_b                (bcast over j)   -> rt = -reduce_n(m1)
      m1 = m1 * e_b                  (bcast over n)   -> -alpha*e*h
      v  = NEGA_b * (-a)_b                            -> +alpha*a
      h += m1 ; h += v
  * preds tail on PE/ACT: rt^T via PE transpose, ft = tanh(Wf1.T@rt^T + F^T),
    pred = sigmoid(W_s.T@ft + b_s), accumulated t-major and transposed at end.
"""
import numpy as np

import concourse.bass as bass
import concourse.bacc as bacc
import concourse.tile as tile
import concourse.mybir as mybir
from concourse import bass_utils

K, D, N, H, V, FT = 50000, 64, 64, 64, 128, 50
B, T = 2048, 128
NCORES = 8
BL = B // NCORES           # 256 batch items per core
NT = BL // 128             # 2 partition tiles per core
ZW = 5 * 64 + FT           # 370 table row width
F32 = mybir.dt.float32
AX = mybir.AxisListType
OP = mybir.AluOpType
AF = mybir.ActivationFunctionType

_CACHE = {}


def _bcast_mid(ap2d, reps):
    """[P, n] -> [P, reps, n]: value repeated along new middle dim."""
    return bass.AP(ap2d.tensor, ap2d.offset, [ap2d.ap[0], [0, reps], ap2d.ap[1]])


def _build_program(b_s_val: float, t_steps: int = T):
    nc = bacc.Bacc("TRN2", target_bir_lowering=False, debug=False,
                   num_devices=NCORES)
    ztab = nc.dram_tensor("ztab", [K, ZW], F32, kind="ExternalInput")
    qidx = nc.dram_tensor("qidx", [NT, 128, T], mybir.dt.int32,
                          kind="ExternalInput")
    score = nc.dram_tensor("score", [NT, 128, T], F32, kind="ExternalInput")
    h0rep = nc.dram_tensor("h0rep", [128, N * H], F32, kind="ExternalInput")
    wf1 = nc.dram_tensor("wf1", [H, FT], F32, kind="ExternalInput")
    ws = nc.dram_tensor("ws", [FT, 1], F32, kind="ExternalInput")
    ident = nc.dram_tensor("ident", [128, 128], F32, kind="ExternalInput")
    hout = nc.dram_tensor("hout", [NT, 128, N * H], F32, kind="ExternalOutput")
    preds = nc.dram_tensor("preds", [NT, T, 128], F32, kind="ExternalOutput")

    JN = N * H  # 4096

    with tile.TileContext(nc) as tc:
        with (
            tc.tile_pool(name="state", bufs=1) as st,
            tc.tile_pool(name="zp", bufs=3) as zp,
            tc.tile_pool(name="sp", bufs=3) as sp,
            tc.tile_pool(name="mp", bufs=2) as mp,
            tc.tile_pool(name="tailp", bufs=2) as tp,
            tc.tile_pool(name="ps2", bufs=2, space="PSUM") as ps2,
        ):
            # ---- persistent state / constants ----
            hs = [st.tile([128, JN], F32, tag=f"h{i}", name=f"h{i}")
                  for i in range(NT)]
            qts = [st.tile([128, T], mybir.dt.int32, tag=f"q{i}", name=f"q{i}")
                   for i in range(NT)]
            scs = [st.tile([128, T], F32, tag=f"s{i}", name=f"sc{i}")
                   for i in range(NT)]
            wf1s = st.tile([H, FT], F32, tag="wf1")
            wss = st.tile([FT, 1], F32, tag="ws")
            ids = st.tile([128, 128], F32, tag="id")

            nc.sync.dma_start(wf1s[:], wf1[:])
            nc.sync.dma_start(wss[:], ws[:])
            nc.sync.dma_start(ids[:], ident[:])
            for i in range(NT):
                nc.sync.dma_start(hs[i][:], h0rep[:])
                nc.sync.dma_start(qts[i][:], qidx[i])
                nc.sync.dma_start(scs[i][:], score[i])

            # ---- time loop ----
            for t in range(t_steps):
                for i in range(NT):
                    h = hs[i]
                    # gather Z rows for (all b in tile, t)
                    zt = zp.tile([128, ZW], F32, tag="z")
                    nc.gpsimd.indirect_dma_start(
                        out=zt[:], out_offset=None, in_=ztab[:],
                        in_offset=bass.IndirectOffsetOnAxis(
                            ap=qts[i][:, t:t + 1], axis=0),
                    )
                    sc_t = scs[i][:, t:t + 1]
                    # alpha = softmax(G); store NEGA = -alpha
                    ex = sp.tile([128, N], F32, tag="ex")
                    nc.scalar.activation(ex[:], zt[:, 0:N], AF.Exp)
                    ssum = sp.tile([128, 1], F32, tag="ssum")
                    nc.vector.tensor_reduce(out=ssum[:], in_=ex[:], axis=AX.X,
                                            op=OP.add)
                    rcp = sp.tile([128, 1], F32, tag="rcp")
                    nc.vector.reciprocal(rcp[:], ssum[:])
                    nega = sp.tile([128, N], F32, tag="nega")
                    nc.vector.tensor_scalar(
                        out=nega[:], in0=ex[:], scalar1=rcp[:, :1],
                        scalar2=-1.0, op0=OP.mult, op1=OP.mult)
                    # et = sigmoid(Elo + sc*Ehi) ; nat = -tanh(Alo + sc*Ahi)
                    ue = sp.tile([128, H], F32, tag="ue")
                    nc.vector.scalar_tensor_tensor(
                        out=ue[:], in0=zt[:, 2 * N:3 * N], scalar=sc_t,
                        in1=zt[:, N:2 * N], op0=OP.mult, op1=OP.add)
                    et = sp.tile([128, H], F32, tag="et")
                    nc.scalar.activation(et[:], ue[:], AF.Sigmoid)
                    ua = sp.tile([128, H], F32, tag="ua")
                    nc.vector.scalar_tensor_tensor(
                        out=ua[:], in0=zt[:, 4 * N:5 * N], scalar=sc_t,
                        in1=zt[:, 3 * N:4 * N], op0=OP.mult, op1=OP.add)
                    nat = sp.tile([128, H], F32, tag="nat")
                    nc.scalar.activation(nat[:], ua[:], AF.Tanh, scale=-1.0)

                    # broadcast views
                    nega_b = _bcast_mid(nega[:], H)          # [p, j~, n]
                    et_b = et[:].to_broadcast([128, H, N])   # [p, j, n~]
                    nat_b = nat[:].to_broadcast([128, H, N])
                    h3 = h[:].rearrange("p (j n) -> p j n", n=N)

                    # scan update
                    m1 = mp.tile([128, JN], F32, tag="m1")
                    m13 = m1[:].rearrange("p (j n) -> p j n", n=N)
                    nc.vector.tensor_tensor(out=m13, in0=h3, in1=nega_b,
                                            op=OP.mult)
                    rt = sp.tile([128, H], F32, tag="rt")
                    nc.vector.tensor_reduce(out=rt[:], in_=m13, axis=AX.X,
                                            op=OP.add, negate=True)
                    v = mp.tile([128, JN], F32, tag="v")
                    v3 = v[:].rearrange("p (j n) -> p j n", n=N)
                    nc.gpsimd.tensor_tensor(out=v3, in0=nega_b, in1=nat_b,
                                            op=OP.mult)
                    nc.vector.tensor_tensor(out=m13, in0=m13, in1=et_b,
                                            op=OP.mult)
                    nc.vector.tensor_tensor(out=h[:], in0=h[:], in1=m1[:],
                                            op=OP.add)
                    nc.gpsimd.tensor_tensor(out=h[:], in0=h[:], in1=v[:],
                                            op=OP.add)

                    # preds tail: ftT = tanh(Wf1.T @ rt.T + F.T)
                    rtT = ps2.tile([H, 128], F32, tag="rtT", space="PSUM")
                    nc.tensor.matmul(rtT[:], lhsT=rt[:], rhs=ids[:],
                                     is_transpose=True, start=True, stop=True)
                    rtTs = tp.tile([H, 128], F32, tag="rtTs")
                    nc.scalar.copy(rtTs[:], rtT[:])
                    ftT = ps2.tile([FT, 128], F32, tag="ftT", space="PSUM")
                    nc.tensor.matmul(ftT[:], lhsT=wf1s[:], rhs=rtTs[:],
                                     start=True, stop=False)
                    nc.tensor.matmul(ftT[:], lhsT=zt[:, 5 * N:5 * N + FT],
                                     rhs=ids[:], is_transpose=True,
                                     start=False, stop=True)
                    ftTs = tp.tile([FT, 128], F32, tag="ftTs")
                    nc.scalar.activation(ftTs[:], ftT[:], AF.Tanh)
                    pp = ps2.tile([1, 128], F32, tag="pp", space="PSUM")
                    nc.tensor.matmul(pp[:], lhsT=wss[:], rhs=ftTs[:],
                                     start=True, stop=True)
                    prow = tp.tile([1, 128], F32, tag="prow")
                    nc.scalar.activation(prow[:], pp[:],
                                         AF.Sigmoid, bias=float(b_s_val))
                    nc.sync.dma_start(preds[i, t], prow[:])

            # ---- outputs ----
            for i in range(NT):
                nc.sync.dma_start(hout[i], hs[i][:])
    nc.compile()
    return nc


def _prep_shared(knowledge_memory, W_k, b_k, W_ft, b_ft, W_s, b_s,
                 W_cks, b_cks, W_e, b_e, W_a, b_a):
    f = np.float32
    km = np.asarray(knowledge_memory, f)
    W_k = np.asarray(W_k, f); b_k = np.asarray(b_k, f)
    W_ft = np.asarray(W_ft, f); b_ft = np.asarray(b_ft, f)
    W_cks = np.asarray(W_cks, f); b_cks = np.asarray(b_cks, f)
    W_e = np.asarray(W_e, f); b_e = np.asarray(b_e, f)
    W_a = np.asarray(W_a, f); b_a = np.asarray(b_a, f)

    G = W_k @ km.T + (b_k @ km.T)[None, :]
    Elo = W_cks[:K] @ W_e + (b_cks @ W_e + b_e)[None, :]
    Ehi = W_cks[K:] @ W_e
    Alo = W_cks[:K] @ W_a + (b_cks @ W_a + b_a)[None, :]
    Ahi = W_cks[K:] @ W_a
    F = W_k @ W_ft[H:] + (b_k @ W_ft[H:] + b_ft)[None, :]
    Z = np.ascontiguousarray(
        np.concatenate([G, Elo, Ehi, Alo, Ahi, F], axis=1), dtype=f)
    return Z


def kernel(q_idx, score, knowledge_memory, h_initial,
           W_k, b_k, W_ft, b_ft, W_s, b_s,
           W_cks, b_cks, W_e, b_e, W_a, b_a):
    f = np.float32
    q_idx = np.asarray(q_idx)
    score = np.asarray(score, f)
    h_initial = np.asarray(h_initial, f)
    W_ft = np.asarray(W_ft, f)
    W_s = np.asarray(W_s, f)
    b_s_val = float(np.asarray(b_s, f))

    Z = _prep_shared(knowledge_memory, W_k, b_k, W_ft, b_ft, W_s, b_s,
                     W_cks, b_cks, W_e, b_e, W_a, b_a)
    h0rep = np.ascontiguousarray(
        np.broadcast_to(h_initial.T.reshape(-1), (128, N * H)), dtype=f)
    wf1 = np.ascontiguousarray(W_ft[:H], f)
    ws = np.ascontiguousarray(W_s.reshape(FT, 1), f)
    ident = np.eye(128, dtype=f)

    if "nc" not in _CACHE:
        _CACHE["nc"] = _build_program(b_s_val)
    nc = _CACHE["nc"]

    qi = np.ascontiguousarray(q_idx.reshape(NCORES, NT, 128, T), np.int32)
    sc = np.ascontiguousarray(score.reshape(NCORES, NT, 128, T), f)
    in_maps = []
    for c in range(NCORES):
        in_maps.append(dict(ztab=Z, qidx=qi[c], score=sc[c], h0rep=h0rep,
                            wf1=wf1, ws=ws, ident=ident))

    res = bass_utils.run_bass_kernel_spmd(nc, in_maps,
                                          core_ids=list(range(NCORES)))
    preds_full = np.empty((B, T), f)
    h_final = np.empty((B, N, H), f)
    for c in range(NCORES):
        r = res.results[c]
        preds_full[c * BL:(c + 1) * BL] = (
            r["preds"].reshape(NT, T, 128).transpose(0, 2, 1).reshape(BL, T))
        h_final[c * BL:(c + 1) * BL] = (
            r["hout"].reshape(BL, H, N).transpose(0, 2, 1))
    return preds_full, h_final


# revision 10
# speedup vs baseline: 22.8284x; 22.8284x over previous
"""DKVMN Bass kernel for Trainium2 (8 NeuronCores, data-parallel over batch).

Math (per batch item b, per step t):
    kn    = W_k[idx] + b_k
    alpha = softmax(kn @ km.T)                       [N]
    rt    = sum_n alpha[n] * h[n, :]                 [H]
    ft    = tanh([rt, kn] @ W_ft + b_ft);  pred = sigmoid(ft @ W_s + b_s)
    vt    = W_cks[idx] + sc * W_cks[idx+K] + b_cks
    et    = sigmoid(vt @ W_e + b_e);  at = tanh(vt @ W_a + b_a)
    h     = h * (1 - alpha (x) et) + alpha (x) at

Key restructuring:
  * All per-index quantities are gathered from HOST-premultiplied tables:
      G   = W_k @ km.T   (+ b_k@km.T)          -> softmax logits rows
      Elo/Ehi = W_cks[:K]@W_e / W_cks[K:]@W_e  (+ bias fold)
      Alo/Ahi = W_cks[:K]@W_a / W_cks[K:]@W_a  (+ bias fold)
      F   = W_k @ W_ft[H:] (+ b_k@W_ft[H:] + b_ft)
    packed as one row-gatherable table Z [K, 370].
  * The recurrent state h is SBUF-resident in layout h[b, j*64+n] (j=hidden,
    n=slot).  Per step, with NEGA = -alpha:
      m1 = h * NEGA_b                (bcast over j)   -> rt = -reduce_n(m1)
      m1 = m1 * e_b                  (bcast over n)   -> -alpha*e*h
      v  = NEGA_b * (-a)_b                            -> +alpha*a
      h += m1 ; h += v
  * preds tail on PE/ACT: rt^T via PE transpose, ft = tanh(Wf1.T@rt^T + F^T),
    pred = sigmoid(W_s.T@ft + b_s), accumulated t-major and transposed at end.
"""
import numpy as np

import concourse.bass as bass
import concourse.bacc as bacc
import concourse.tile as tile
import concourse.mybir as mybir
from concourse import bass_utils

K, D, N, H, V, FT = 50000, 64, 64, 64, 128, 50
B, T = 2048, 128
NCORES = 8
BL = B // NCORES           # 256 batch items per core
NT = BL // 128             # 2 partition tiles per core
ZW = 5 * 64 + FT           # 370 table row width
F32 = mybir.dt.float32
AX = mybir.AxisListType
OP = mybir.AluOpType
AF = mybir.ActivationFunctionType

_CACHE = {}


def _bcast_mid(ap2d, reps):
    """[P, n] -> [P, reps, n]: value repeated along new middle dim."""
    return bass.AP(ap2d.tensor, ap2d.offset, [ap2d.ap[0], [0, reps], ap2d.ap[1]])


def _build_program(b_s_val: float, t_steps: int = T):
    nc = bacc.Bacc("TRN2", target_bir_lowering=False, debug=False,
                   num_devices=NCORES)
    ztab = nc.dram_tensor("ztab", [K, ZW], F32, kind="ExternalInput")
    qidx = nc.dram_tensor("qidx", [NT, 128, T], mybir.dt.int32,
                          kind="ExternalInput")
    score = nc.dram_tensor("score", [NT, 128, T], F32, kind="ExternalInput")
    h0rep = nc.dram_tensor("h0rep", [128, N * H], F32, kind="ExternalInput")
    wf1 = nc.dram_tensor("wf1", [H, FT], F32, kind="ExternalInput")
    ws = nc.dram_tensor("ws", [FT, 1], F32, kind="ExternalInput")
    ident = nc.dram_tensor("ident", [128, 128], F32, kind="ExternalInput")
    hout = nc.dram_tensor("hout", [NT, 128, N * H], F32, kind="ExternalOutput")
    preds = nc.dram_tensor("preds", [NT, T, 128], F32, kind="ExternalOutput")

    JN = N * H  # 4096

    with tile.TileContext(nc) as tc:
        with (
            tc.tile_pool(name="state", bufs=1) as st,
            tc.tile_pool(name="zp", bufs=3) as zp,
            tc.tile_pool(name="sp", bufs=3) as sp,
            tc.tile_pool(name="mp", bufs=2) as mp,
            tc.tile_pool(name="tailp", bufs=2) as tp,
            tc.tile_pool(name="ps2", bufs=2, space="PSUM") as ps2,
        ):
            # ---- persistent state / constants ----
            hs = [st.tile([128, JN], F32, tag=f"h{i}", name=f"h{i}")
                  for i in range(NT)]
            qts = [st.tile([128, T], mybir.dt.int32, tag=f"q{i}", name=f"q{i}")
                   for i in range(NT)]
            scs = [st.tile([128, T], F32, tag=f"s{i}", name=f"sc{i}")
                   for i in range(NT)]
            wf1s = st.tile([H, FT], F32, tag="wf1")
            wss = st.tile([FT, 1], F32, tag="ws")
            ids = st.tile([128, 128], F32, tag="id")

            nc.sync.dma_start(wf1s[:], wf1[:])
            nc.sync.dma_start(wss[:], ws[:])
            nc.sync.dma_start(ids[:], ident[:])
            for i in range(NT):
                nc.sync.dma_start(hs[i][:], h0rep[:])
                nc.sync.dma_start(qts[i][:], qidx[i])
                nc.sync.dma_start(scs[i][:], score[i])

            # ---- time loop ----
            for t in range(t_steps):
                for i in range(NT):
                    h = hs[i]
                    # gather Z rows for (all b in tile, t)
                    zt = zp.tile([128, ZW], F32, tag="z")
                    nc.gpsimd.indirect_dma_start(
                        out=zt[:], out_offset=None, in_=ztab[:],
                        in_offset=bass.IndirectOffsetOnAxis(
                            ap=qts[i][:, t:t + 1], axis=0),
                    )
                    sc_t = scs[i][:, t:t + 1]
                    # alpha = softmax(G); store NEGA = -alpha
                    ex = sp.tile([128, N], F32, tag="ex")
                    nc.scalar.activation(ex[:], zt[:, 0:N], AF.Exp)
                    ssum = sp.tile([128, 1], F32, tag="ssum")
                    nc.vector.tensor_reduce(out=ssum[:], in_=ex[:], axis=AX.X,
                                            op=OP.add)
                    rcp = sp.tile([128, 1], F32, tag="rcp")
                    nc.vector.reciprocal(rcp[:], ssum[:])
                    nega = sp.tile([128, N], F32, tag="nega")
                    nc.vector.tensor_scalar(
                        out=nega[:], in0=ex[:], scalar1=rcp[:, :1],
                        scalar2=-1.0, op0=OP.mult, op1=OP.mult)
                    # et = sigmoid(Elo + sc*Ehi) ; nat = -tanh(Alo + sc*Ahi)
                    ue = sp.tile([128, H], F32, tag="ue")
                    nc.vector.scalar_tensor_tensor(
                        out=ue[:], in0=zt[:, 2 * N:3 * N], scalar=sc_t,
                        in1=zt[:, N:2 * N], op0=OP.mult, op1=OP.add)
                    et = sp.tile([128, H], F32, tag="et")
                    nc.scalar.activation(et[:], ue[:], AF.Sigmoid)
                    ua = sp.tile([128, H], F32, tag="ua")
                    nc.vector.scalar_tensor_tensor(
                        out=ua[:], in0=zt[:, 4 * N:5 * N], scalar=sc_t,
                        in1=zt[:, 3 * N:4 * N], op0=OP.mult, op1=OP.add)
                    nat = sp.tile([128, H], F32, tag="nat")
                    nc.scalar.activation(nat[:], ua[:], AF.Tanh, scale=-1.0)

                    # broadcast views
                    nega_b = _bcast_mid(nega[:], H)          # [p, j~, n]
                    et_b = et[:].to_broadcast([128, H, N])   # [p, j, n~]
                    h3 = h[:].rearrange("p (j n) -> p j n", n=N)

                    # scan update:  m1 = h*(-alpha); rt = sum_n alpha*h;
                    # m1 *= e (-> -alpha*e*h); v = alpha*a; h += m1; h += v.
                    m1 = mp.tile([128, JN], F32, tag="m1")
                    m13 = m1[:].rearrange("p (j n) -> p j n", n=N)
                    nc.vector.tensor_tensor(out=m13, in0=h3, in1=nega_b,
                                            op=OP.mult)
                    rt = sp.tile([128, H], F32, tag="rt")
                    nc.vector.tensor_reduce(out=rt[:], in_=m13, axis=AX.X,
                                            op=OP.add, negate=True)
                    v = mp.tile([128, JN], F32, tag="v")
                    v3 = v[:].rearrange("p (j n) -> p j n", n=N)
                    nat_b = nat[:].to_broadcast([128, H, N])
                    nc.gpsimd.tensor_tensor(out=v3, in0=nega_b, in1=nat_b,
                                            op=OP.mult)
                    nc.vector.tensor_tensor(out=m13, in0=m13, in1=et_b,
                                            op=OP.mult)
                    nc.vector.tensor_tensor(out=h[:], in0=h[:], in1=m1[:],
                                            op=OP.add)
                    nc.gpsimd.tensor_tensor(out=h[:], in0=h[:], in1=v[:],
                                            op=OP.add)

                    # preds tail: ftT = tanh(Wf1.T @ rt.T + F.T)
                    rtT = ps2.tile([H, 128], F32, tag="rtT", space="PSUM")
                    nc.tensor.matmul(rtT[:], lhsT=rt[:], rhs=ids[:],
                                     is_transpose=True, start=True, stop=True)
                    rtTs = tp.tile([H, 128], F32, tag="rtTs")
                    nc.scalar.copy(rtTs[:], rtT[:])
                    ftT = ps2.tile([FT, 128], F32, tag="ftT", space="PSUM")
                    nc.tensor.matmul(ftT[:], lhsT=wf1s[:], rhs=rtTs[:],
                                     start=True, stop=False)
                    nc.tensor.matmul(ftT[:], lhsT=zt[:, 5 * N:5 * N + FT],
                                     rhs=ids[:], is_transpose=True,
                                     start=False, stop=True)
                    ftTs = tp.tile([FT, 128], F32, tag="ftTs")
                    nc.scalar.activation(ftTs[:], ftT[:], AF.Tanh)
                    pp = ps2.tile([1, 128], F32, tag="pp", space="PSUM")
                    nc.tensor.matmul(pp[:], lhsT=wss[:], rhs=ftTs[:],
                                     start=True, stop=True)
                    prow = tp.tile([1, 128], F32, tag="prow")
                    nc.scalar.activation(prow[:], pp[:],
                                         AF.Sigmoid, bias=float(b_s_val))
                    nc.sync.dma_start(preds[i, t], prow[:])

            # ---- outputs ----
            for i in range(NT):
                nc.sync.dma_start(hout[i], hs[i][:])
    nc.compile()
    return nc


def _prep_shared(knowledge_memory, W_k, b_k, W_ft, b_ft, W_s, b_s,
                 W_cks, b_cks, W_e, b_e, W_a, b_a):
    f = np.float32
    km = np.asarray(knowledge_memory, f)
    W_k = np.asarray(W_k, f); b_k = np.asarray(b_k, f)
    W_ft = np.asarray(W_ft, f); b_ft = np.asarray(b_ft, f)
    W_cks = np.asarray(W_cks, f); b_cks = np.asarray(b_cks, f)
    W_e = np.asarray(W_e, f); b_e = np.asarray(b_e, f)
    W_a = np.asarray(W_a, f); b_a = np.asarray(b_a, f)

    G = W_k @ km.T + (b_k @ km.T)[None, :]
    Elo = W_cks[:K] @ W_e + (b_cks @ W_e + b_e)[None, :]
    Ehi = W_cks[K:] @ W_e
    Alo = W_cks[:K] @ W_a + (b_cks @ W_a + b_a)[None, :]
    Ahi = W_cks[K:] @ W_a
    F = W_k @ W_ft[H:] + (b_k @ W_ft[H:] + b_ft)[None, :]
    Z = np.ascontiguousarray(
        np.concatenate([G, Elo, Ehi, Alo, Ahi, F], axis=1), dtype=f)
    return Z


def kernel(q_idx, score, knowledge_memory, h_initial,
           W_k, b_k, W_ft, b_ft, W_s, b_s,
           W_cks, b_cks, W_e, b_e, W_a, b_a):
    f = np.float32
    q_idx = np.asarray(q_idx)
    score = np.asarray(score, f)
    h_initial = np.asarray(h_initial, f)
    W_ft = np.asarray(W_ft, f)
    W_s = np.asarray(W_s, f)
    b_s_val = float(np.asarray(b_s, f))

    Z = _prep_shared(knowledge_memory, W_k, b_k, W_ft, b_ft, W_s, b_s,
                     W_cks, b_cks, W_e, b_e, W_a, b_a)
    h0rep = np.ascontiguousarray(
        np.broadcast_to(h_initial.T.reshape(-1), (128, N * H)), dtype=f)
    wf1 = np.ascontiguousarray(W_ft[:H], f)
    ws = np.ascontiguousarray(W_s.reshape(FT, 1), f)
    ident = np.eye(128, dtype=f)

    key = ("nc", b_s_val)
    if key not in _CACHE:
        _CACHE[key] = _build_program(b_s_val)
    nc = _CACHE[key]
    _CACHE["nc"] = nc  # for introspection (test.py timeline)

    qi = np.ascontiguousarray(q_idx.reshape(NCORES, NT, 128, T), np.int32)
    sc = np.ascontiguousarray(score.reshape(NCORES, NT, 128, T), f)
    in_maps = []
    for c in range(NCORES):
        in_maps.append(dict(ztab=Z, qidx=qi[c], score=sc[c], h0rep=h0rep,
                            wf1=wf1, ws=ws, ident=ident))

    res = bass_utils.run_bass_kernel_spmd(nc, in_maps,
                                          core_ids=list(range(NCORES)))
    preds_full = np.empty((B, T), f)
    h_final = np.empty((B, N, H), f)
    for c in range(NCORES):
        r = res.results[c]
        preds_full[c * BL:(c + 1) * BL] = (
            r["preds"].reshape(NT, T, 128).transpose(0, 2, 1).reshape(BL, T))
        h_final[c * BL:(c + 1) * BL] = (
            r["hout"].reshape(BL, H, N).transpose(0, 2, 1))
    return preds_full, h_final
